# revision 1
# baseline (speedup 1.0000x reference)
"""Two-layer GCN encoder (GCNConv x2 + minmax + L2 normalize) on 8 TRN2 NeuronCores.

Sharding: nodes row-partitioned across 8 cores (12500/core); each edge owned by the
core owning its destination. Per core, edges are grouped by 128-node destination
block and by source chunk (25000-row table chunks keep dma_gather's int16 indices
in range), padded to 128-edge tiles; per-(block,chunk) tile counts are equalized
across cores so one SPMD program serves all 8.

Per layer: the (N x d) linear-transform table is computed shard-wise, AllGathered,
then each superblock of 4 destination blocks issues one dma_gather per source
chunk (128-row batches of 512B/256B rows). Aggregation on TensorE: per 128-edge
tile a selection matrix S[e,j] = norm[e] * (dst_local[e]==j) is built with one
fused DVE tensor_scalar (is_equal then mult vs an iota row); layer 1 accumulates
Msg^T @ S (transposed, so the +b1 bias and the h@W2 lhsT need no transpose),
layer 2 accumulates S^T @ Msg node-major, followed by minmax + L2 normalize.
"""

import math

import numpy as np

import concourse.bass as bass
import concourse.bacc as bacc
import concourse.mybir as mybir
import concourse.tile as tile
from concourse import bass_utils

NCORES = 8
BLK = 128
IN_C = 128
HID = 128
OUT_C = 64
CHUNK_ROWS = 25000  # dma_gather idx is int16: chunk-relative indices < 32768
SBN = 4  # destination blocks per gather superblock

LAST_RESULTS = None
_PROGRAM_CACHE = {}


def _host_prep(x, edge_index):
    n = x.shape[0]
    assert n % NCORES == 0
    npc = n // NCORES
    nblk = math.ceil(npc / BLK)
    n_chunks = math.ceil(n / CHUNK_ROWS)

    src = edge_index[0].astype(np.int64)
    dst = edge_index[1].astype(np.int64)

    deg = (np.bincount(dst, minlength=n) + 1).astype(np.float32)
    dinv = (1.0 / np.sqrt(deg)).astype(np.float32)

    loop = np.arange(n, dtype=np.int64)
    s_all = np.concatenate([src, loop])
    d_all = np.concatenate([dst, loop])
    norm_all = (dinv[s_all] * dinv[d_all]).astype(np.float32)

    core = d_all // npc
    within = d_all % npc
    blk = within // BLK
    colv = (within % BLK).astype(np.float32)
    chunk = s_all // CHUNK_ROWS

    key = (core * nblk + blk) * n_chunks + chunk
    counts = np.bincount(key, minlength=NCORES * nblk * n_chunks).reshape(
        NCORES, nblk * n_chunks
    )
    # tiles per (block, chunk), equalized across cores (SPMD)
    t4 = ((counts + BLK - 1) // BLK).max(axis=0).reshape(nblk, n_chunks)

    # global tile order: for each superblock: for each chunk: for each block: tiles
    gofs = np.zeros((nblk, n_chunks), np.int64)
    cur = 0
    n_sb = math.ceil(nblk / SBN)
    for sbi in range(n_sb):
        for q in range(n_chunks):
            for b in range(sbi * SBN, min((sbi + 1) * SBN, nblk)):
                gofs[b, q] = cur
                cur += int(t4[b, q])
    t_total = cur

    order = np.argsort(key, kind="stable")
    ks = key[order]
    ss = s_all[order]
    cs = colv[order]
    nn = norm_all[order]

    group_start = np.zeros(NCORES * nblk * n_chunks, np.int64)
    group_start[1:] = np.cumsum(counts.ravel())[:-1]
    r = np.arange(len(ks), dtype=np.int64) - group_start[ks]
    t_idx = r // BLK
    p_idx = r % BLK
    c_idx = ks // (nblk * n_chunks)
    b_idx = (ks // n_chunks) % nblk
    q_idx = ks % n_chunks
    gcol = gofs[b_idx, q_idx] + t_idx
    rel = (ss - q_idx * CHUNK_ROWS).astype(np.int16)

    # int16 idx stream for dma_gather: index k of a call lives at
    # [k%16 (+16*replica), call_col0*8 + k//16]; with 128-multiple groups this
    # reduces to [p%16, gcol*8 + p//16] independent of call boundaries.
    srcs16 = np.zeros((NCORES, 16, t_total * 8), np.int16)
    dstf_arr = np.zeros((NCORES, BLK, t_total), np.float32)
    normf_arr = np.zeros((NCORES, BLK, t_total), np.float32)
    srcs16[c_idx, p_idx % 16, gcol * 8 + p_idx // 16] = rel
    dstf_arr[c_idx, p_idx, gcol] = cs
    normf_arr[c_idx, p_idx, gcol] = nn
    srcs16 = np.tile(srcs16, (1, 8, 1))  # replicate for the 8 Q7 cores

    xt = np.ascontiguousarray(x.T.astype(np.float32))
    in_maps = []
    for c in range(NCORES):
        in_maps.append(
            {
                "xT": np.ascontiguousarray(xt[:, c * npc : (c + 1) * npc]),
                "srcs16": np.ascontiguousarray(srcs16[c]),
                "dstf": np.ascontiguousarray(dstf_arr[c]),
                "normf": np.ascontiguousarray(normf_arr[c]),
            }
        )
    return in_maps, t4, gofs, npc, nblk, n_chunks


def _build_nc(n, npc, nblk, n_chunks, t4, gofs):
    t_total = int(t4.sum())
    f32 = mybir.dt.float32
    i16 = mybir.dt.int16
    n_sb = math.ceil(nblk / SBN)

    nc = bacc.Bacc(
        "TRN2",
        target_bir_lowering=False,
        debug=False,
        enable_asserts=False,
        num_devices=NCORES,
    )

    xT = nc.dram_tensor("xT", [IN_C, npc], f32, kind="ExternalInput").ap()
    W1 = nc.dram_tensor("W1", [IN_C, HID], f32, kind="ExternalInput").ap()
    W2 = nc.dram_tensor("W2", [HID, OUT_C], f32, kind="ExternalInput").ap()
    b1c = nc.dram_tensor("b1c", [HID, 1], f32, kind="ExternalInput").ap()
    b2b = nc.dram_tensor("b2b", [BLK, OUT_C], f32, kind="ExternalInput").ap()
    iota = nc.dram_tensor("iota", [BLK, BLK], f32, kind="ExternalInput").ap()
    srcs16 = nc.dram_tensor(
        "srcs16", [BLK, t_total * 8], i16, kind="ExternalInput"
    ).ap()
    dstf = nc.dram_tensor("dstf", [BLK, t_total], f32, kind="ExternalInput").ap()
    normf = nc.dram_tensor("normf", [BLK, t_total], f32, kind="ExternalInput").ap()
    out = nc.dram_tensor("out", [npc, OUT_C], f32, kind="ExternalOutput").ap()

    ieq = mybir.AluOpType.is_equal
    mul = mybir.AluOpType.mult
    sub = mybir.AluOpType.subtract

    def nb_of(b):
        return min(BLK, npc - b * BLK)

    def sb_blocks(sbi):
        return range(sbi * SBN, min((sbi + 1) * SBN, nblk))

    with tile.TileContext(nc) as tc:
        with (
            tc.tile_pool(name="dram", bufs=1, space="DRAM") as dram,
            tc.tile_pool(name="const", bufs=1) as constp,
            tc.tile_pool(name="meta", bufs=1) as metap,
            tc.tile_pool(name="io", bufs=3) as iop,
            tc.tile_pool(name="idx", bufs=2) as idxp,
            tc.tile_pool(name="msg", bufs=2) as msgp,
            tc.tile_pool(name="sel", bufs=4) as selp,
            tc.tile_pool(name="fin", bufs=2) as finp,
            tc.tile_pool(name="stat", bufs=3) as statp,
            tc.tile_pool(name="psA", bufs=2, space="PSUM") as psA,
            tc.tile_pool(name="psB", bufs=2, space="PSUM") as psB,
        ):
            xw1_shard = dram.tile([npc, HID], f32)
            xw1_full = dram.tile([n, HID], f32, addr_space="Shared")
            hw2_shard = dram.tile([npc, OUT_C], f32)
            hw2_full = dram.tile([n, OUT_C], f32, addr_space="Shared")

            W1s = constp.tile([IN_C, HID], f32)
            nc.sync.dma_start(out=W1s[:], in_=W1)
            W2s = constp.tile([HID, OUT_C], f32)
            nc.sync.dma_start(out=W2s[:], in_=W2)
            b1s = constp.tile([HID, 1], f32)
            nc.sync.dma_start(out=b1s[:], in_=b1c)
            b2s = constp.tile([BLK, OUT_C], f32)
            nc.sync.dma_start(out=b2s[:], in_=b2b)
            iotas = constp.tile([BLK, BLK], f32)
            nc.sync.dma_start(out=iotas[:], in_=iota)
            dstf_s = metap.tile([BLK, t_total], f32)
            nc.sync.dma_start(out=dstf_s[:], in_=dstf)
            normf_s = metap.tile([BLK, t_total], f32)
            nc.sync.dma_start(out=normf_s[:], in_=normf)

            # ---- Phase 0: xw1_shard = x_c @ W1 ----
            XCH = 4
            for bc in range(0, nblk, XCH):
                hi = min(bc + XCH, nblk)
                w = min(hi * BLK, npc) - bc * BLK
                xt_t = iop.tile([IN_C, XCH * BLK], f32, tag="xt")
                nc.sync.dma_start(out=xt_t[:, :w], in_=xT[:, bc * BLK : bc * BLK + w])
                for b in range(bc, hi):
                    nb = nb_of(b)
                    o = (b - bc) * BLK
                    ps = psA.tile([BLK, HID], f32, tag="psA")
                    nc.tensor.matmul(
                        out=ps[:nb, :],
                        lhsT=xt_t[:, o : o + nb],
                        rhs=W1s[:],
                        start=True,
                        stop=True,
                    )
                    xw_t = iop.tile([BLK, HID], f32, tag="xw")
                    nc.scalar.copy(xw_t[:nb, :], ps[:nb, :])
                    nc.sync.dma_start(
                        out=xw1_shard[b * BLK : b * BLK + nb, :], in_=xw_t[:nb, :]
                    )

            nc.gpsimd.collective_compute(
                "AllGather",
                mybir.AluOpType.bypass,
                replica_groups=[list(range(NCORES))],
                ins=[xw1_shard[:]],
                outs=[xw1_full[:]],
            )

            def gather_sb(sbi, table_full, elem, msg_tag):
                """One superblock's gathers: returns (msg tile, sb_col0, T_sb)."""
                blocks = list(sb_blocks(sbi))
                sb_col0 = int(gofs[blocks[0], 0])
                t_sb = int(sum(t4[b, q] for b in blocks for q in range(n_chunks)))
                idx_t = idxp.tile([BLK, t_sb * 8], i16, tag="idx")
                nc.sync.dma_start(
                    out=idx_t[:],
                    in_=srcs16[:, sb_col0 * 8 : (sb_col0 + t_sb) * 8],
                )
                msg = msgp.tile([BLK, t_sb * elem], f32, tag=msg_tag)
                MAXT = 8  # dma_gather caps at 1024 indices per call
                for q in range(n_chunks):
                    cs = int(gofs[blocks[0], q])
                    tq = int(sum(t4[b, q] for b in blocks))
                    hi_r = min((q + 1) * CHUNK_ROWS, n)
                    for k in range(0, tq, MAXT):
                        tk = min(MAXT, tq - k)
                        lo = cs - sb_col0 + k
                        nc.gpsimd.dma_gather(
                            out_ap=msg[:, lo * elem : (lo + tk) * elem].rearrange(
                                "p (t e) -> p t e", e=elem
                            ),
                            in_ap=table_full[q * CHUNK_ROWS : hi_r, :],
                            idxs_ap=idx_t[:, lo * 8 : (lo + tk) * 8],
                            num_idxs=tk * BLK,
                            num_idxs_reg=tk * BLK,
                            elem_size=elem,
                        )
                return msg, sb_col0, t_sb

            def block_tiles(b):
                return [
                    int(gofs[b, q] + t)
                    for q in range(n_chunks)
                    for t in range(int(t4[b, q]))
                ]

            # ---- Phase 1+2 ----
            for sbi in range(n_sb):
                msg, sb_col0, _ = gather_sb(sbi, xw1_full, HID, "msg")
                for b in sb_blocks(sbi):
                    nb = nb_of(b)
                    cols = block_tiles(b)
                    psT = psA.tile([BLK, HID], f32, tag="psA")
                    for j, g in enumerate(cols):
                        S = selp.tile([BLK, BLK], f32, tag="S")
                        nc.vector.tensor_scalar(
                            out=S[:],
                            in0=iotas[:],
                            scalar1=dstf_s[:, g : g + 1],
                            scalar2=normf_s[:, g : g + 1],
                            op0=ieq,
                            op1=mul,
                        )
                        lo = g - sb_col0
                        nc.tensor.matmul(
                            out=psT[:, :nb],
                            lhsT=msg[:, lo * HID : (lo + 1) * HID],
                            rhs=S[:, :nb],
                            start=(j == 0),
                            stop=(j == len(cols) - 1),
                        )
                    hT = finp.tile([HID, BLK], f32, tag="hT")
                    nc.vector.tensor_scalar_add(hT[:, :nb], psT[:, :nb], b1s[:])
                    ps2 = psB.tile([BLK, OUT_C], f32, tag="psB")
                    nc.tensor.matmul(
                        out=ps2[:nb, :],
                        lhsT=hT[:, :nb],
                        rhs=W2s[:],
                        start=True,
                        stop=True,
                    )
                    hw2_t = finp.tile([BLK, OUT_C], f32, tag="hw2")
                    nc.scalar.copy(hw2_t[:nb, :], ps2[:nb, :])
                    nc.sync.dma_start(
                        out=hw2_shard[b * BLK : b * BLK + nb, :], in_=hw2_t[:nb, :]
                    )

            nc.gpsimd.collective_compute(
                "AllGather",
                mybir.AluOpType.bypass,
                replica_groups=[list(range(NCORES))],
                ins=[hw2_shard[:]],
                outs=[hw2_full[:]],
            )

            # ---- Phase 3 ----
            for sbi in range(n_sb):
                msg2, sb_col0, _ = gather_sb(sbi, hw2_full, OUT_C, "msg")
                for b in sb_blocks(sbi):
                    nb = nb_of(b)
                    cols = block_tiles(b)
                    psO = psB.tile([BLK, OUT_C], f32, tag="psB")
                    for j, g in enumerate(cols):
                        S = selp.tile([BLK, BLK], f32, tag="S")
                        nc.vector.tensor_scalar(
                            out=S[:],
                            in0=iotas[:],
                            scalar1=dstf_s[:, g : g + 1],
                            scalar2=normf_s[:, g : g + 1],
                            op0=ieq,
                            op1=mul,
                        )
                        lo = g - sb_col0
                        nc.tensor.matmul(
                            out=psO[:nb, :],
                            lhsT=S[:, :nb],
                            rhs=msg2[:, lo * OUT_C : (lo + 1) * OUT_C],
                            start=(j == 0),
                            stop=(j == len(cols) - 1),
                        )
                    z = finp.tile([BLK, OUT_C], f32, tag="z")
                    nc.vector.tensor_add(z[:nb, :], psO[:nb, :], b2s[:nb, :])
                    zmax = statp.tile([BLK, 1], f32, tag="zmax")
                    nc.vector.tensor_reduce(
                        zmax[:nb], z[:nb, :], axis=mybir.AxisListType.X,
                        op=mybir.AluOpType.max,
                    )
                    zmin = statp.tile([BLK, 1], f32, tag="zmin")
                    nc.vector.tensor_reduce(
                        zmin[:nb], z[:nb, :], axis=mybir.AxisListType.X,
                        op=mybir.AluOpType.min,
                    )
                    rng_t = statp.tile([BLK, 1], f32, tag="rng")
                    nc.vector.tensor_sub(rng_t[:nb], zmax[:nb], zmin[:nb])
                    rinv = statp.tile([BLK, 1], f32, tag="rinv")
                    nc.vector.reciprocal(rinv[:nb], rng_t[:nb])
                    zs = finp.tile([BLK, OUT_C], f32, tag="zs")
                    nc.vector.tensor_scalar(
                        out=zs[:nb, :],
                        in0=z[:nb, :],
                        scalar1=zmin[:nb],
                        scalar2=rinv[:nb],
                        op0=sub,
                        op1=mul,
                    )
                    sq = finp.tile([BLK, OUT_C], f32, tag="sq")
                    ssq = statp.tile([BLK, 1], f32, tag="ssq")
                    nc.scalar.activation(
                        sq[:nb, :],
                        zs[:nb, :],
                        mybir.ActivationFunctionType.Square,
                        accum_out=ssq[:nb],
                    )
                    snrm = statp.tile([BLK, 1], f32, tag="snrm")
                    nc.scalar.sqrt(snrm[:nb], ssq[:nb])
                    nc.vector.tensor_scalar_max(snrm[:nb], snrm[:nb], 1e-12)
                    ninv = statp.tile([BLK, 1], f32, tag="ninv")
                    nc.vector.reciprocal(ninv[:nb], snrm[:nb])
                    res = finp.tile([BLK, OUT_C], f32, tag="res")
                    nc.vector.tensor_scalar_mul(res[:nb, :], zs[:nb, :], ninv[:nb])
                    nc.sync.dma_start(
                        out=out[b * BLK : b * BLK + nb, :], in_=res[:nb, :]
                    )

    nc.compile()
    return nc


def kernel(x, edge_index, W1, b1, W2, b2, trace=False):
    global LAST_RESULTS
    x = np.asarray(x)
    edge_index = np.asarray(edge_index)
    W1 = np.asarray(W1, dtype=np.float32)
    b1 = np.asarray(b1, dtype=np.float32)
    W2 = np.asarray(W2, dtype=np.float32)
    b2 = np.asarray(b2, dtype=np.float32)

    n = x.shape[0]
    in_maps, t4, gofs, npc, nblk, n_chunks = _host_prep(x, edge_index)

    consts = {
        "W1": np.ascontiguousarray(W1),
        "W2": np.ascontiguousarray(W2),
        "b1c": np.ascontiguousarray(b1.reshape(HID, 1)),
        "b2b": np.ascontiguousarray(np.tile(b2.reshape(1, OUT_C), (BLK, 1))),
        "iota": np.tile(np.arange(BLK, dtype=np.float32), (BLK, 1)),
    }
    for m in in_maps:
        m.update(consts)

    key = (n, t4.tobytes())
    nc = _PROGRAM_CACHE.get(key)
    if nc is None:
        nc = _build_nc(n, npc, nblk, n_chunks, t4, gofs)
        _PROGRAM_CACHE[key] = nc

    results = bass_utils.run_bass_kernel_spmd(
        nc, in_maps, core_ids=list(range(NCORES)), trace=trace
    )
    LAST_RESULTS = results
    return np.concatenate([results.results[c]["out"] for c in range(NCORES)], axis=0)



# revision 12
# speedup vs baseline: 1.6024x; 1.6024x over previous
"""Two-layer GCN encoder (GCNConv x2 + minmax + L2 normalize) on 8 TRN2 NeuronCores.

Sharding: nodes row-partitioned across 8 cores (12500/core); each edge owned by the
core owning its destination. Per core, edges are grouped by 128-node destination
block and by source chunk (25000-row table chunks keep dma_gather's int16 indices
in range), padded to 128-edge tiles; per-(block,chunk) tile counts are equalized
across cores so one SPMD program serves all 8.

v3: message tables are bf16 (256B gather rows); gathers run on 4 SWDGE queues
rotated per call so the Q7 descriptor generation of one call overlaps the DMA
drain of the previous three. Layer 2 aggregates the bf16 H1 rows first and
applies W2 to the aggregate afterwards (linearity), so no second linear table is
built; layer 1 aggregates node-major (lhsT=S), layer 2 feature-major (lhsT=Msg),
so neither layer transposes. Per tile the selection matrix S[e,j]=norm[e]*
(dst[e]==j) is built either on DVE (one fused is_equal*mult tensor_scalar) or on
the otherwise-idle Activation engine (Square(iota-dst) then Relu(norm-norm*t^2)),
load-balancing the two engines. Biases ride the PE as rank-1 matmuls.
"""

import math

import numpy as np
from ml_dtypes import bfloat16

import concourse.bass as bass
import concourse.bacc as bacc
import concourse.mybir as mybir
import concourse.tile as tile
from concourse import bass_utils

NCORES = 8
BLK = 128
IN_C = 128
HID = 128
OUT_C = 64
CHUNK_ROWS = 25000  # dma_gather idx is int16: chunk-relative indices < 32768
SBN = 8  # destination blocks per gather superblock
MAXT = 8  # dma_gather caps at 1024 indices per call
NQ = 4  # SWDGE queues, rotated per gather call
ACT_FRAC = 0.4  # fraction of S-builds on the Activation engine

LAST_RESULTS = None
_PROGRAM_CACHE = {}
DEBUG_DUMPS = False


def _host_prep(x, edge_index):
    n = x.shape[0]
    assert n % NCORES == 0
    npc = n // NCORES
    nblk = math.ceil(npc / BLK)
    n_chunks = math.ceil(n / CHUNK_ROWS)

    src = edge_index[0].astype(np.int64)
    dst = edge_index[1].astype(np.int64)

    deg = (np.bincount(dst, minlength=n) + 1).astype(np.float32)
    dinv = (1.0 / np.sqrt(deg)).astype(np.float32)

    loop = np.arange(n, dtype=np.int64)
    s_all = np.concatenate([src, loop])
    d_all = np.concatenate([dst, loop])
    norm_all = (dinv[s_all] * dinv[d_all]).astype(np.float32)

    core = d_all // npc
    within = d_all % npc
    blk = within // BLK
    colv = (within % BLK).astype(np.float32)
    chunk = s_all // CHUNK_ROWS

    key = (core * nblk + blk) * n_chunks + chunk
    counts = np.bincount(key, minlength=NCORES * nblk * n_chunks).reshape(
        NCORES, nblk * n_chunks
    )
    # tiles per (block, chunk), equalized across cores (SPMD)
    t4 = ((counts + BLK - 1) // BLK).max(axis=0).reshape(nblk, n_chunks)

    # global tile order: for each superblock: for each chunk: for each block: tiles
    gofs = np.zeros((nblk, n_chunks), np.int64)
    cur = 0
    n_sb = math.ceil(nblk / SBN)
    for sbi in range(n_sb):
        for q in range(n_chunks):
            for b in range(sbi * SBN, min((sbi + 1) * SBN, nblk)):
                gofs[b, q] = cur
                cur += int(t4[b, q])
    t_total = cur

    order = np.argsort(key, kind="stable")
    ks = key[order]
    ss = s_all[order]
    cs = colv[order]
    nn = norm_all[order]

    group_start = np.zeros(NCORES * nblk * n_chunks, np.int64)
    group_start[1:] = np.cumsum(counts.ravel())[:-1]
    r = np.arange(len(ks), dtype=np.int64) - group_start[ks]
    t_idx = r // BLK
    p_idx = r % BLK
    c_idx = ks // (nblk * n_chunks)
    b_idx = (ks // n_chunks) % nblk
    q_idx = ks % n_chunks
    gcol = gofs[b_idx, q_idx] + t_idx
    rel = (ss - q_idx * CHUNK_ROWS).astype(np.int16)

    # int16 idx stream for dma_gather: index k of a call lives at
    # [k%16 (+16*replica), call_col0*8 + k//16]; with 128-multiple groups this
    # reduces to [p%16, gcol*8 + p//16] independent of call boundaries.
    srcs16 = np.zeros((NCORES, 16, t_total * 8), np.int16)
    dstf_arr = np.zeros((NCORES, BLK, t_total), np.float32)
    normf_arr = np.zeros((NCORES, BLK, t_total), np.float32)
    srcs16[c_idx, p_idx % 16, gcol * 8 + p_idx // 16] = rel
    dstf_arr[c_idx, p_idx, gcol] = cs
    normf_arr[c_idx, p_idx, gcol] = nn
    srcs16 = np.tile(srcs16, (1, 8, 1))  # replicate for the 8 Q7 cores

    xt = np.ascontiguousarray(x.T.astype(bfloat16))
    in_maps = []
    for c in range(NCORES):
        in_maps.append(
            {
                "xT": np.ascontiguousarray(xt[:, c * npc : (c + 1) * npc]),
                "srcs16": np.ascontiguousarray(srcs16[c]),
                "dstf": np.ascontiguousarray(dstf_arr[c]),
                "ndstf": np.ascontiguousarray(-dstf_arr[c]),
                "normf": np.ascontiguousarray(normf_arr[c]),
                "nnormf": np.ascontiguousarray(-normf_arr[c]),
            }
        )
    return in_maps, t4, gofs, npc, nblk, n_chunks


def _build_nc(n, npc, nblk, n_chunks, t4, gofs):
    t_total = int(t4.sum())
    f32 = mybir.dt.float32
    bf16 = mybir.dt.bfloat16
    i16 = mybir.dt.int16
    n_sb = math.ceil(nblk / SBN)

    nc = bacc.Bacc(
        "TRN2",
        target_bir_lowering=False,
        debug=False,
        enable_asserts=False,
        num_devices=NCORES,
        num_swdge_queues=NQ,
    )

    xT = nc.dram_tensor("xT", [IN_C, npc], bf16, kind="ExternalInput").ap()
    W1 = nc.dram_tensor("W1", [IN_C, HID], bf16, kind="ExternalInput").ap()
    W2 = nc.dram_tensor("W2", [HID, OUT_C], bf16, kind="ExternalInput").ap()
    b1r = nc.dram_tensor("b1r", [1, HID], bf16, kind="ExternalInput").ap()
    b2r = nc.dram_tensor("b2r", [1, OUT_C], bf16, kind="ExternalInput").ap()
    iota = nc.dram_tensor("iota", [BLK, BLK], bf16, kind="ExternalInput").ap()
    srcs16 = nc.dram_tensor(
        "srcs16", [BLK, t_total * 8], i16, kind="ExternalInput"
    ).ap()
    dstf = nc.dram_tensor("dstf", [BLK, t_total], f32, kind="ExternalInput").ap()
    ndstf = nc.dram_tensor("ndstf", [BLK, t_total], f32, kind="ExternalInput").ap()
    normf = nc.dram_tensor("normf", [BLK, t_total], f32, kind="ExternalInput").ap()
    nnormf = nc.dram_tensor(
        "nnormf", [BLK, t_total], f32, kind="ExternalInput"
    ).ap()
    out = nc.dram_tensor("out", [npc, OUT_C], f32, kind="ExternalOutput").ap()

    ieq = mybir.AluOpType.is_equal
    mul = mybir.AluOpType.mult
    sub = mybir.AluOpType.subtract
    AF = mybir.ActivationFunctionType

    def nb_of(b):
        return min(BLK, npc - b * BLK)

    def sb_blocks(sbi):
        return range(sbi * SBN, min((sbi + 1) * SBN, nblk))

    qctr = [0]
    actr = [0]

    with tile.TileContext(nc) as tc:
        with (
            tc.tile_pool(name="dram", bufs=1, space="DRAM") as dram,
            tc.tile_pool(name="const", bufs=1) as constp,
            tc.tile_pool(name="meta", bufs=1) as metap,
            tc.tile_pool(name="io", bufs=3) as iop,
            tc.tile_pool(name="idx", bufs=2) as idxp,
            tc.tile_pool(name="msg", bufs=3) as msgp,
            tc.tile_pool(name="sel", bufs=6) as selp,
            tc.tile_pool(name="fin", bufs=3) as finp,
            tc.tile_pool(name="stat", bufs=3) as statp,
            tc.tile_pool(name="psA", bufs=2, space="PSUM") as psA,
            tc.tile_pool(name="psB", bufs=2, space="PSUM") as psB,
        ):
            xw1_shard = dram.tile([npc, HID], bf16)
            xw1_full = dram.tile([n, HID], bf16, addr_space="Shared")
            h1_shard = dram.tile([npc, HID], bf16)
            h1_full = dram.tile([n, HID], bf16, addr_space="Shared")

            W1s = constp.tile([IN_C, HID], bf16)
            nc.sync.dma_start(out=W1s[:], in_=W1)
            W2s = constp.tile([HID, OUT_C], bf16)
            nc.sync.dma_start(out=W2s[:], in_=W2)
            b1s = constp.tile([1, HID], bf16)
            nc.sync.dma_start(out=b1s[:], in_=b1r)
            b2s = constp.tile([1, OUT_C], bf16)
            nc.sync.dma_start(out=b2s[:], in_=b2r)
            ones1 = constp.tile([1, BLK], bf16)
            nc.vector.memset(ones1[:], 1.0)
            iotas = constp.tile([BLK, BLK], bf16)
            nc.sync.dma_start(out=iotas[:], in_=iota)
            dstf_s = metap.tile([BLK, t_total], f32)
            nc.sync.dma_start(out=dstf_s[:], in_=dstf)
            ndstf_s = metap.tile([BLK, t_total], f32)
            nc.sync.dma_start(out=ndstf_s[:], in_=ndstf)
            normf_s = metap.tile([BLK, t_total], f32)
            nc.sync.dma_start(out=normf_s[:], in_=normf)
            nnormf_s = metap.tile([BLK, t_total], f32)
            nc.sync.dma_start(out=nnormf_s[:], in_=nnormf)

            # ---- Phase 0: xw1_shard = bf16(x_c @ W1) ----
            XCH = 4
            for bc in range(0, nblk, XCH):
                hi = min(bc + XCH, nblk)
                w = min(hi * BLK, npc) - bc * BLK
                xt_t = iop.tile([IN_C, XCH * BLK], bf16, tag="xt")
                nc.sync.dma_start(out=xt_t[:, :w], in_=xT[:, bc * BLK : bc * BLK + w])
                for b in range(bc, hi):
                    nb = nb_of(b)
                    o = (b - bc) * BLK
                    ps = psA.tile([BLK, HID], f32, tag="psA")
                    nc.tensor.matmul(
                        out=ps[:nb, :],
                        lhsT=xt_t[:, o : o + nb],
                        rhs=W1s[:],
                        start=True,
                        stop=True,
                    )
                    xw_t = iop.tile([BLK, HID], bf16, tag="xw")
                    nc.scalar.copy(xw_t[:nb, :], ps[:nb, :])
                    nc.sync.dma_start(
                        out=xw1_shard[b * BLK : b * BLK + nb, :], in_=xw_t[:nb, :]
                    )

            nc.gpsimd.collective_compute(
                "AllGather",
                mybir.AluOpType.bypass,
                replica_groups=[list(range(NCORES))],
                ins=[xw1_shard[:]],
                outs=[xw1_full[:]],
            )
            if DEBUG_DUMPS:
                dxw = nc.dram_tensor(
                    "dbg_xw1_full", [n, HID], bf16, kind="ExternalOutput"
                ).ap()
                nc.sync.dma_start(out=dxw, in_=xw1_full[:])

            def gather_sb(sbi, table_full, msg_tag):
                """One superblock's gathers: returns (msg tile, sb_col0)."""
                blocks = list(sb_blocks(sbi))
                sb_col0 = int(gofs[blocks[0], 0])
                t_sb = int(sum(t4[b, q] for b in blocks for q in range(n_chunks)))
                idx_t = idxp.tile([BLK, t_sb * 8], i16, tag="idx")
                nc.sync.dma_start(
                    out=idx_t[:],
                    in_=srcs16[:, sb_col0 * 8 : (sb_col0 + t_sb) * 8],
                )
                msg = msgp.tile([BLK, t_sb * HID], bf16, tag=msg_tag)
                for q in range(n_chunks):
                    cs = int(gofs[blocks[0], q])
                    tq = int(sum(t4[b, q] for b in blocks))
                    hi_r = min((q + 1) * CHUNK_ROWS, n)
                    for k in range(0, tq, MAXT):
                        tk = min(MAXT, tq - k)
                        lo = cs - sb_col0 + k
                        nc.gpsimd.dma_gather(
                            out_ap=msg[:, lo * HID : (lo + tk) * HID].rearrange(
                                "p (t e) -> p t e", e=HID
                            ),
                            in_ap=table_full[q * CHUNK_ROWS : hi_r, :],
                            idxs_ap=idx_t[:, lo * 8 : (lo + tk) * 8],
                            num_idxs=tk * BLK,
                            num_idxs_reg=tk * BLK,
                            elem_size=HID,
                            queue_num=qctr[0] % NQ,
                        )
                        qctr[0] += 1
                return msg, sb_col0

            def build_S(g):
                """Selection matrix for tile g, alternating DVE / ACT."""
                S = selp.tile([BLK, BLK], bf16, tag="S")
                actr[0] += 1
                if (actr[0] % 100) < 100 * ACT_FRAC:
                    t2 = selp.tile([BLK, BLK], bf16, tag="t2")
                    nc.scalar.activation(
                        t2[:], iotas[:], AF.Square, bias=ndstf_s[:, g : g + 1],
                        scale=1.0,
                    )
                    nc.scalar.activation(
                        S[:], t2[:], AF.Relu, bias=normf_s[:, g : g + 1],
                        scale=nnormf_s[:, g : g + 1],
                    )
                else:
                    nc.vector.tensor_scalar(
                        out=S[:],
                        in0=iotas[:],
                        scalar1=dstf_s[:, g : g + 1],
                        scalar2=normf_s[:, g : g + 1],
                        op0=ieq,
                        op1=mul,
                    )
                return S

            def block_tiles(b):
                return [
                    int(gofs[b, q] + t)
                    for q in range(n_chunks)
                    for t in range(int(t4[b, q]))
                ]

            # ---- Phase 1: H1 = bf16(agg(norm * xw1[src]) + b1), node-major ----
            for sbi in range(n_sb):
                msg, sb_col0 = gather_sb(sbi, xw1_full, "msg")
                for b in sb_blocks(sbi):
                    nb = nb_of(b)
                    cols = block_tiles(b)
                    psO = psA.tile([BLK, HID], f32, tag="psA")
                    nc.tensor.matmul(
                        out=psO[:nb, :],
                        lhsT=ones1[:, :nb],
                        rhs=b1s[:],
                        start=True,
                        stop=False,
                    )
                    for j, g in enumerate(cols):
                        S = build_S(g)
                        lo = g - sb_col0
                        nc.tensor.matmul(
                            out=psO[:nb, :],
                            lhsT=S[:, :nb],
                            rhs=msg[:, lo * HID : (lo + 1) * HID],
                            start=False,
                            stop=(j == len(cols) - 1),
                        )
                    h1_t = finp.tile([BLK, HID], bf16, tag="h1")
                    nc.scalar.copy(h1_t[:nb, :], psO[:nb, :])
                    nc.sync.dma_start(
                        out=h1_shard[b * BLK : b * BLK + nb, :], in_=h1_t[:nb, :]
                    )

            nc.gpsimd.collective_compute(
                "AllGather",
                mybir.AluOpType.bypass,
                replica_groups=[list(range(NCORES))],
                ins=[h1_shard[:]],
                outs=[h1_full[:]],
            )
            if DEBUG_DUMPS:
                dh1 = nc.dram_tensor(
                    "dbg_h1_full", [n, HID], bf16, kind="ExternalOutput"
                ).ap()
                nc.sync.dma_start(out=dh1, in_=h1_full[:])

            # ---- Phase 2: z = agg(norm * h1[src]) @ W2 + b2, then minmax + L2 ----
            for sbi in range(n_sb):
                msg2, sb_col0 = gather_sb(sbi, h1_full, "msg")
                for b in sb_blocks(sbi):
                    nb = nb_of(b)
                    cols = block_tiles(b)
                    psT = psA.tile([BLK, BLK], f32, tag="psA")
                    for j, g in enumerate(cols):
                        S = build_S(g)
                        lo = g - sb_col0
                        nc.tensor.matmul(
                            out=psT[:, :nb],
                            lhsT=msg2[:, lo * HID : (lo + 1) * HID],
                            rhs=S[:, :nb],
                            start=(j == 0),
                            stop=(j == len(cols) - 1),
                        )
                    aggT = finp.tile([HID, BLK], bf16, tag="aggT")
                    nc.scalar.copy(aggT[:, :nb], psT[:, :nb])
                    ps2 = psB.tile([BLK, OUT_C], f32, tag="psB")
                    nc.tensor.matmul(
                        out=ps2[:nb, :],
                        lhsT=ones1[:, :nb],
                        rhs=b2s[:],
                        start=True,
                        stop=False,
                    )
                    nc.tensor.matmul(
                        out=ps2[:nb, :],
                        lhsT=aggT[:, :nb],
                        rhs=W2s[:],
                        start=False,
                        stop=True,
                    )
                    zmax = statp.tile([BLK, 1], f32, tag="zmax")
                    nc.vector.tensor_reduce(
                        zmax[:nb], ps2[:nb, :], axis=mybir.AxisListType.X,
                        op=mybir.AluOpType.max,
                    )
                    zmin = statp.tile([BLK, 1], f32, tag="zmin")
                    nc.vector.tensor_reduce(
                        zmin[:nb], ps2[:nb, :], axis=mybir.AxisListType.X,
                        op=mybir.AluOpType.min,
                    )
                    rng_t = statp.tile([BLK, 1], f32, tag="rng")
                    nc.vector.tensor_sub(rng_t[:nb], zmax[:nb], zmin[:nb])
                    rinv = statp.tile([BLK, 1], f32, tag="rinv")
                    nc.vector.reciprocal(rinv[:nb], rng_t[:nb])
                    zs = finp.tile([BLK, OUT_C], f32, tag="zs")
                    nc.vector.tensor_scalar(
                        out=zs[:nb, :],
                        in0=ps2[:nb, :],
                        scalar1=zmin[:nb],
                        scalar2=rinv[:nb],
                        op0=sub,
                        op1=mul,
                    )
                    sq = finp.tile([BLK, OUT_C], f32, tag="sq")
                    ssq = statp.tile([BLK, 1], f32, tag="ssq")
                    nc.scalar.activation(
                        sq[:nb, :],
                        zs[:nb, :],
                        AF.Square,
                        accum_out=ssq[:nb],
                    )
                    snrm = statp.tile([BLK, 1], f32, tag="snrm")
                    nc.scalar.sqrt(snrm[:nb], ssq[:nb])
                    nc.vector.tensor_scalar_max(snrm[:nb], snrm[:nb], 1e-12)
                    ninv = statp.tile([BLK, 1], f32, tag="ninv")
                    nc.vector.reciprocal(ninv[:nb], snrm[:nb])
                    res = finp.tile([BLK, OUT_C], f32, tag="res")
                    nc.vector.tensor_scalar_mul(res[:nb, :], zs[:nb, :], ninv[:nb])
                    nc.sync.dma_start(
                        out=out[b * BLK : b * BLK + nb, :], in_=res[:nb, :]
                    )

    nc.compile()
    return nc


def kernel(x, edge_index, W1, b1, W2, b2, trace=False):
    global LAST_RESULTS
    x = np.asarray(x)
    edge_index = np.asarray(edge_index)
    W1 = np.asarray(W1, dtype=np.float32)
    b1 = np.asarray(b1, dtype=np.float32)
    W2 = np.asarray(W2, dtype=np.float32)
    b2 = np.asarray(b2, dtype=np.float32)

    n = x.shape[0]
    in_maps, t4, gofs, npc, nblk, n_chunks = _host_prep(x, edge_index)

    consts = {
        "W1": np.ascontiguousarray(W1.astype(bfloat16)),
        "W2": np.ascontiguousarray(W2.astype(bfloat16)),
        "b1r": np.ascontiguousarray(b1.reshape(1, HID).astype(bfloat16)),
        "b2r": np.ascontiguousarray(b2.reshape(1, OUT_C).astype(bfloat16)),
        "iota": np.tile(np.arange(BLK, dtype=bfloat16), (BLK, 1)),
    }
    for m in in_maps:
        m.update(consts)

    key = (n, t4.tobytes())
    nc = _PROGRAM_CACHE.get(key)
    if nc is None:
        nc = _build_nc(n, npc, nblk, n_chunks, t4, gofs)
        _PROGRAM_CACHE[key] = nc
    results = bass_utils.run_bass_kernel_spmd(
        nc, in_maps, core_ids=list(range(NCORES)), trace=trace
    )
    LAST_RESULTS = results
    return np.concatenate([results.results[c]["out"] for c in range(NCORES)], axis=0)
